# revision 1
# baseline (speedup 1.0000x reference)
"""Conditional NT-Xent loss kernel for Trainium2 (8 NeuronCores, SPMD data-parallel).

Math per chunk of 2 consecutive rows from each of zjs/zis (a,b = zjs rows; c,d = zis rows):
  need squared norms na..nd and 6 pairwise dots s_ab..s_bc of the 4 vectors,
  cos_xy = s_xy / (|x||y|), logits = 2*cos, per-row loss = lse(3 logits) - pos,
  total = sum over rows / B.

Pipeline (V2): cast-DMA loads f32->bf16, PE transposes to feature-major,
DVE/ACT materialize PSUM->SBUF, DVE bf16 products, ACT squares, PE ones-matmul
chunk-major reductions into PSUM, small elementwise epilogue.

Sharding: batch(chunk)-parallel across 8 cores; each core computes a partial
sum [128,1]; host sums partials and divides by B.
"""

import numpy as np

import concourse.bass as bass
import concourse.tile as tile
from concourse import bacc, masks, mybir
from concourse.bass_utils import run_bass_kernel_spmd

N_CORES = 8
B_FULL = 65536            # total rows in zis (== zjs)
ROWS = B_FULL // N_CORES  # 8192 rows per core shard
D = 256
GROUPS = ROWS // 256      # 32 groups of 128 chunks per core
HALF_G = GROUPS // 2
F32 = mybir.dt.float32
BF16 = mybir.dt.bfloat16
ALU = mybir.AluOpType
ACTF = mybir.ActivationFunctionType

# value-type column order per group in the stats tile S[:, g*10 + t]
# t: 0=na 1=nb 2=nc 3=nd 4=s_ab 5=s_cd 6=s_ac 7=s_bd 8=s_ad 9=s_bc
N_VALS = 10

# product pair -> (src_j, k_in0, src_i, k_in1): slice s uses k+s
# src: 0 = TJ (zjs: a=k0/1, b=k2/3), 1 = TI (zis: c=k0/1, d=k2/3)
PAIRS = [
    (0, 0, 0, 2),  # ab
    (1, 0, 1, 2),  # cd
    (0, 0, 1, 0),  # ac
    (0, 2, 1, 2),  # bd
    (0, 0, 1, 2),  # ad
    (0, 2, 1, 0),  # bc
]


def _epilogue(tc, nc, epi, S, out):
    """Per-chunk softmax math on the stats tile S [128, GROUPS*N_VALS] -> out [128,1]."""
    Sv = S[:].rearrange("p (g t) -> p g t", t=N_VALS)
    norms = Sv[:, :, 0:4]
    svals = Sv[:, :, 4:10]

    LN = epi.tile([128, GROUPS * 4], F32, tag="ln")
    LNv = LN[:].rearrange("p (g t) -> p g t", t=4)
    nc.scalar.activation(LNv, norms, ACTF.Ln)

    # q_xy = ln nx + ln ny (pair order ab cd ac bd ad bc)
    Q = epi.tile([128, GROUPS * 6], F32, tag="q")
    Qv = Q[:].rearrange("p (g t) -> p g t", t=6)
    pair_norm_idx = [(0, 1), (2, 3), (0, 2), (1, 3), (0, 3), (1, 2)]
    for t, (x, y) in enumerate(pair_norm_idx):
        nc.vector.tensor_add(
            Qv[:, :, t : t + 1], LNv[:, :, x : x + 1], LNv[:, :, y : y + 1]
        )

    # rr_xy = exp(-0.5 q) = 1/(|x||y|)
    RQ = epi.tile([128, GROUPS * 6], F32, tag="rq")
    RQv = RQ[:].rearrange("p (g t) -> p g t", t=6)
    nc.scalar.activation(RQv, Qv, ACTF.Exp, scale=-0.5)

    # cos_xy = s_xy * rr_xy
    C = epi.tile([128, GROUPS * 6], F32, tag="cos")
    Cv = C[:].rearrange("p (g t) -> p g t", t=6)
    nc.vector.tensor_mul(Cv, svals, RQv)

    # E_xy = exp(2 cos)
    E = epi.tile([128, GROUPS * 6], F32, tag="e")
    Ev = E[:].rearrange("p (g t) -> p g t", t=6)
    nc.scalar.activation(Ev, Cv, ACTF.Exp, scale=2.0)

    def ecol(t):
        return Ev[:, :, t : t + 1]

    # softmax denominators for the 4 rows of each chunk
    DEN = epi.tile([128, GROUPS * 4], F32, tag="den")
    DENv = DEN[:].rearrange("p (g t) -> p g t", t=4)
    TMP = epi.tile([128, GROUPS * 4], F32, tag="tmp")
    TMPv = TMP[:].rearrange("p (g t) -> p g t", t=4)
    den_terms = [
        (0, 4, 2),  # D0 = (ab + ad) + ac
        (0, 5, 3),  # D1 = (ab + bc) + bd
        (5, 1, 2),  # D2 = (bc + cd) + ac
        (4, 1, 3),  # D3 = (ad + cd) + bd
    ]
    for r, (u, v, w) in enumerate(den_terms):
        nc.vector.tensor_add(TMPv[:, :, r : r + 1], ecol(u), ecol(v))
        nc.vector.tensor_add(DENv[:, :, r : r + 1], TMPv[:, :, r : r + 1], ecol(w))

    LD = epi.tile([128, GROUPS * 4], F32, tag="ld")
    LDv = LD[:].rearrange("p (g t) -> p g t", t=4)
    nc.scalar.activation(LDv, DENv, ACTF.Ln)

    LG = epi.tile([128, GROUPS], F32, tag="lg")
    nc.vector.reduce_sum(
        LG[:].rearrange("p (g o) -> p g o", o=1), LDv, axis=mybir.AxisListType.X
    )

    T1 = epi.tile([128, GROUPS], F32, tag="t1")
    nc.vector.tensor_add(
        T1[:].rearrange("p (g o) -> p g o", o=1), Cv[:, :, 2:3], Cv[:, :, 3:4]
    )

    # loss per chunk-col = LG - 4*T1
    LC = epi.tile([128, GROUPS], F32, tag="lc")
    nc.vector.scalar_tensor_tensor(
        out=LC[:], in0=T1[:], scalar=-4.0, in1=LG[:], op0=ALU.mult, op1=ALU.add
    )

    ACC = epi.tile([128, 1], F32, tag="acc")
    nc.vector.reduce_sum(ACC[:], LC[:], axis=mybir.AxisListType.X)
    nc.sync.dma_start(out=out, in_=ACC[:])


def _trace_kernel(tc, nc, zjs, zis, out):
    # quarter-granular natural views: [8 quarters, 128 chunks, 4 groups x 512]
    GQ = 4  # groups per quarter-tile
    zjs_q = zjs.rearrange("(q g p two) f -> q p g (two f)", p=128, two=2, g=GQ)
    zis_q = zis.rearrange("(q g p two) f -> q p g (two f)", p=128, two=2, g=GQ)

    with (
        tc.tile_pool(name="consts", bufs=1) as consts,
        tc.tile_pool(name="loads", bufs=3) as loads,
        tc.tile_pool(name="tpose", bufs=2, space="PSUM") as tpose,
        tc.tile_pool(name="tmat", bufs=2) as tmat,
        tc.tile_pool(name="prod", bufs=2) as prod,
        tc.tile_pool(name="sq", bufs=2) as sqp,
        tc.tile_pool(name="spred", bufs=2, space="PSUM") as spred,
        tc.tile_pool(name="stats", bufs=1) as stats,
        tc.tile_pool(name="epi", bufs=1) as epi,
    ):
        ident = consts.tile([128, 128], BF16)
        masks.make_identity(nc, ident[:])
        ones = consts.tile([128, 1], BF16)
        nc.gpsimd.memset(ones[:], 1.0)

        S = stats.tile([128, GROUPS * N_VALS], F32)

        BLK_G = 8  # groups per compute block
        for blk in range(GROUPS // BLK_G):
            # transposed, feature-major block: cols = (g_local, k_slice, chunk)
            TJ = tmat.tile([128, BLK_G * 512], BF16, tag="tj")
            TI = tmat.tile([128, BLK_G * 512], BF16, tag="ti")

            for q in range(BLK_G // GQ):
                qg = blk * (BLK_G // GQ) + q
                # one big cast-DMA per quarter per input (f32 DRAM -> bf16 SBUF)
                ZJ = loads.tile([128, GQ * 512], BF16, tag="zj")
                nc.gpsimd.dma_start(
                    out=ZJ[:].rearrange("p (g f) -> p g f", g=GQ), in_=zjs_q[qg]
                )
                ZI = loads.tile([128, GQ * 512], BF16, tag="zi")
                nc.gpsimd.dma_start(
                    out=ZI[:].rearrange("p (g f) -> p g f", g=GQ), in_=zis_q[qg]
                )

                for gq in range(GQ):
                    gl = q * GQ + gq
                    PJ = tpose.tile([128, 512], BF16, tag="pj")
                    PI = tpose.tile([128, 512], BF16, tag="pi")
                    for b in range(4):
                        sl = slice(128 * b, 128 * (b + 1))
                        nsl = slice(512 * gq + 128 * b, 512 * gq + 128 * (b + 1))
                        nc.tensor.transpose(PJ[:, sl], ZJ[:, nsl], ident[:])
                        nc.tensor.transpose(PI[:, sl], ZI[:, nsl], ident[:])

                    cs = slice(512 * gl, 512 * (gl + 1))
                    # split PSUM->SBUF materialize between DVE and ACT
                    if gl % 4 == 3:
                        nc.vector.tensor_copy(TJ[:, cs], PJ[:])
                        nc.vector.tensor_copy(TI[:, cs], PI[:])
                    else:
                        nc.vector.tensor_copy(TJ[:, cs], PJ[:])
                        nc.scalar.activation(TI[:, cs], PI[:], ACTF.Copy)

            TJv = TJ[:].rearrange("p (g k c) -> p g k c", k=4, c=128)
            TIv = TI[:].rearrange("p (g k c) -> p g k c", k=4, c=128)
            srcs = (TJv, TIv)

            # products: 6 pairs x 2 feature-slices, [128, HALF_G*128] bf16 each
            ptiles = []
            for pi_, (s0, k0, s1, k1) in enumerate(PAIRS):
                slices = []
                for s in range(2):
                    P = prod.tile([128, BLK_G * 128], BF16, tag=f"p{pi_}s{s}")
                    Pv = P[:].rearrange("p (g o c) -> p g o c", o=1, c=128)
                    nc.vector.tensor_mul(
                        Pv,
                        srcs[s0][:, :, k0 + s : k0 + s + 1, :],
                        srcs[s1][:, :, k1 + s : k1 + s + 1, :],
                    )
                    slices.append(P)
                ptiles.append(slices)

            # squares for the norms
            SQJ = sqp.tile([128, BLK_G * 512], BF16, tag="sqj")
            nc.scalar.activation(SQJ[:], TJ[:], ACTF.Square)
            SQI = sqp.tile([128, BLK_G * 512], BF16, tag="sqi")
            nc.scalar.activation(SQI[:], TI[:], ACTF.Square)
            sqs = (SQJ, SQI)

            # chunk-major reductions: out[chunk, 1] = sum over 128 feats
            SP = spred.tile([128, BLK_G * N_VALS], F32, tag="sp")
            for gl in range(BLK_G):
                base = 512 * gl
                for t in range(4):  # na nb nc nd
                    sq_t = sqs[t // 2]
                    koff = 256 * (t % 2)
                    for s in range(2):
                        lh = sq_t[:, base + koff + 128 * s : base + koff + 128 * (s + 1)]
                        nc.tensor.matmul(
                            SP[:, N_VALS * gl + t : N_VALS * gl + t + 1],
                            lh,
                            ones[:, 0:1],
                            start=(s == 0),
                            stop=(s == 1),
                        )
                for pi_ in range(6):
                    t = 4 + pi_
                    for s in range(2):
                        lh = ptiles[pi_][s][:, 128 * gl : 128 * (gl + 1)]
                        nc.tensor.matmul(
                            SP[:, N_VALS * gl + t : N_VALS * gl + t + 1],
                            lh,
                            ones[:, 0:1],
                            start=(s == 0),
                            stop=(s == 1),
                        )

            # drain the half's stats to SBUF
            nc.vector.tensor_copy(
                S[:, blk * BLK_G * N_VALS : (blk + 1) * BLK_G * N_VALS], SP[:]
            )

        _epilogue(tc, nc, epi, S, out)


_NC_CACHE = None


def _build_nc():
    global _NC_CACHE
    if _NC_CACHE is not None:
        return _NC_CACHE
    nc = bacc.Bacc("TRN2", target_bir_lowering=False, debug=False, num_devices=N_CORES)
    zjs = nc.dram_tensor("zjs", [ROWS, D], F32, kind="ExternalInput")
    zis = nc.dram_tensor("zis", [ROWS, D], F32, kind="ExternalInput")
    out = nc.dram_tensor("out", [128, 1], F32, kind="ExternalOutput")
    with tile.TileContext(nc) as tc:
        _trace_kernel(tc, nc, zjs.ap(), zis.ap(), out.ap())
    nc.compile()
    _NC_CACHE = nc
    return nc


def run_cores(zis, zjs, trace=False):
    """Run the SPMD kernel; returns (list of per-core out arrays, BassKernelResults)."""
    nc = _build_nc()
    zis = np.ascontiguousarray(np.asarray(zis, dtype=np.float32))
    zjs = np.ascontiguousarray(np.asarray(zjs, dtype=np.float32))
    in_maps = []
    for i in range(N_CORES):
        sl = slice(i * ROWS, (i + 1) * ROWS)
        in_maps.append({"zis": zis[sl], "zjs": zjs[sl]})
    res = run_bass_kernel_spmd(nc, in_maps, list(range(N_CORES)), trace=trace)
    return [r["out"] for r in res.results], res


def kernel(zis, zjs):
    outs, _ = run_cores(zis, zjs, trace=False)
    total = np.sum([o.astype(np.float64).sum() for o in outs])
    return np.asarray(total / B_FULL, dtype=np.float32)



# revision 3
# speedup vs baseline: 1.0407x; 1.0407x over previous
"""Conditional NT-Xent loss kernel for Trainium2 (8 NeuronCores, SPMD data-parallel).

Per chunk t: a=zjs[2t], b=zjs[2t+1], c=zis[2t], d=zis[2t+1].
Needs 4 norms (na..nd) + 6 dots (ab cd ac bd ad bc), then per-row softmax math.

Pipeline per block of 4 groups (group = 128 chunks):
  - gpsimd cast-DMA f32->bf16 chunk-major loads (graded sizes, few instrs)
  - both tensors -> feature-major PSUM via PE transposes (software-pipelined
    ahead of the reduce-matmuls so the in-order PE queue never stalls)
  - zis materialized PSUM->SBUF (alternating DVE/ACT) so every DVE product
    has at most one PSUM operand (hardware restriction)
  - DVE bf16 products: TJxTI -> [ac|bd], TJxTI(neg-stride swap) -> [ad|bc],
    cd, and ab via an a-slice SBUF copy; ACT squares for the norms
  - PE ones-matmul column reductions (2 accumulating matmuls per stat) into
    one persistent PSUM stats bank
  - epilogue: DVE fast-inverse-sqrt (+1 Newton) for 1/sqrt(nx*ny) so ACT only
    needs Exp (table set 0, shared with Square) plus one final Ln (set 5)
Host sums the 8 cores' [128,1] partials / B.
"""

import numpy as np

import concourse.bass as bass
import concourse.tile as tile
from concourse import bacc, masks, mybir
from concourse.bass_utils import run_bass_kernel_spmd

N_CORES = 8
B_FULL = 65536
ROWS = B_FULL // N_CORES  # 8192
D = 256
GROUPS = ROWS // 256      # 32 groups of 128 chunks
BLK_G = 4                 # groups per pipeline block
N_BLKS = GROUPS // BLK_G
HALF_BLKS = N_BLKS // 2
F32 = mybir.dt.float32
BF16 = mybir.dt.bfloat16
FP8 = mybir.dt.float8e4
ALU = mybir.AluOpType
ACTF = mybir.ActivationFunctionType

N_VALS = 10  # stat col order per group: na nb nc nd ab cd ac bd ad bc


MAGIC_F = float(np.frombuffer(np.uint32(0x5F3759DF).tobytes(), dtype=np.float32)[0])
I32 = mybir.dt.int32


def _epi_pre(tc, nc, epi, S, half, full=False):
    """Epilogue up to the denominators: DVE math + one ACT Exp
    (table set 0, same as Square - safe to interleave with the main loop)."""
    G = GROUPS if full else GROUPS // 2
    Sv = S.rearrange("p (g t) -> p g t", t=N_VALS)
    svals = Sv[:, :, 4:10]

    # RP_xy = nx * ny (pair order ab cd ac bd ad bc)
    RP = epi.tile([128, G * 6], F32, name=f"rp{half}", tag=f"rp{half}")
    RPv = RP[:].rearrange("p (g t) -> p g t", t=6)
    pair_norm_idx = [(0, 1), (2, 3), (0, 2), (1, 3), (0, 3), (1, 2)]
    for t, (x, y) in enumerate(pair_norm_idx):
        nc.vector.tensor_mul(
            RPv[:, :, t : t + 1], Sv[:, :, x : x + 1], Sv[:, :, y : y + 1]
        )

    # rr = rsqrt(RP) via fast-inverse-sqrt + 1 Newton iteration (DVE only)
    MAG = epi.tile([128, G * 6], F32, name=f"mag{half}", tag=f"mag{half}")
    nc.gpsimd.memset(MAG[:], MAGIC_F)
    C15 = epi.tile([128, G * 6], F32, name=f"c15{half}", tag=f"c15{half}")
    nc.gpsimd.memset(C15[:], 1.5)
    Y = epi.tile([128, G * 6], F32, name=f"y{half}", tag=f"y{half}")
    SH = epi.tile([128, G * 6], F32, name=f"sh{half}", tag=f"sh{half}")
    nc.vector.tensor_scalar(
        out=SH[:].bitcast(I32),
        in0=RP[:].bitcast(I32),
        scalar1=1,
        scalar2=None,
        op0=ALU.logical_shift_right,
    )
    nc.vector.tensor_tensor(
        out=Y[:].bitcast(I32),
        in0=MAG[:].bitcast(I32),
        in1=SH[:].bitcast(I32),
        op=ALU.subtract,
    )
    T2 = epi.tile([128, G * 6], F32, name=f"t2{half}", tag=f"t2{half}")
    T3 = epi.tile([128, G * 6], F32, name=f"t3{half}", tag=f"t3{half}")
    nc.vector.tensor_mul(T2[:], Y[:], Y[:])
    nc.vector.tensor_mul(T2[:], T2[:], RP[:])
    nc.vector.scalar_tensor_tensor(
        out=T3[:], in0=T2[:], scalar=-0.5, in1=C15[:], op0=ALU.mult, op1=ALU.add
    )
    nc.vector.tensor_mul(Y[:], Y[:], T3[:])

    # cos_xy = s_xy * rr_xy
    C = epi.tile([128, G * 6], F32, name=f"cos{half}", tag=f"cos{half}")
    Cv = C[:].rearrange("p (g t) -> p g t", t=6)
    nc.vector.tensor_mul(Cv, svals, Y[:].rearrange("p (g t) -> p g t", t=6))

    # E_xy = exp(2 cos)  (ACT set 0, same as Square)
    E = epi.tile([128, G * 6], F32, name=f"e{half}", tag=f"e{half}")
    Ev = E[:].rearrange("p (g t) -> p g t", t=6)
    nc.scalar.activation(Ev, Cv, ACTF.Exp, scale=2.0)

    def ecol(t):
        return Ev[:, :, t : t + 1]

    # softmax denominators for the 4 rows of each chunk
    DEN = epi.tile([128, G * 4], F32, name=f"den{half}", tag=f"den{half}")
    DENv = DEN[:].rearrange("p (g t) -> p g t", t=4)
    TMP = epi.tile([128, G * 4], F32, name=f"tmp{half}", tag=f"tmp{half}")
    TMPv = TMP[:].rearrange("p (g t) -> p g t", t=4)
    den_terms = [
        (0, 4, 2),  # D0 = (ab + ad) + ac
        (0, 5, 3),  # D1 = (ab + bc) + bd
        (5, 1, 2),  # D2 = (bc + cd) + ac
        (4, 1, 3),  # D3 = (ad + cd) + bd
    ]
    for r, (u, v, w) in enumerate(den_terms):
        nc.vector.tensor_add(TMPv[:, :, r : r + 1], ecol(u), ecol(v))
        nc.vector.tensor_add(DENv[:, :, r : r + 1], TMPv[:, :, r : r + 1], ecol(w))
    return DEN, C


def _epi_post2(tc, nc, epi, den, cos, out):
    """Final Ln (single ACT set switch) + loss accumulation."""
    LD = epi.tile([128, GROUPS * 4], F32, name="ld", tag="ld")
    nc.scalar.activation(LD[:], den[:], ACTF.Ln)

    LG = epi.tile([128, GROUPS], F32, name="lg", tag="lg")
    nc.vector.reduce_sum(
        LG[:].rearrange("p (g o) -> p g o", o=1),
        LD[:].rearrange("p (g t) -> p g t", t=4),
        axis=mybir.AxisListType.X,
    )

    T1 = epi.tile([128, GROUPS], F32, name="t1", tag="t1")
    T1v = T1[:].rearrange("p (g o) -> p g o", o=1)
    Cv = cos[:].rearrange("p (g t) -> p g t", t=6)
    nc.vector.tensor_add(T1v[:], Cv[:, :, 2:3], Cv[:, :, 3:4])

    # loss per chunk-col = LG - 4*T1
    LC = epi.tile([128, GROUPS], F32, name="lc", tag="lc")
    nc.vector.scalar_tensor_tensor(
        out=LC[:], in0=T1[:], scalar=-4.0, in1=LG[:], op0=ALU.mult, op1=ALU.add
    )

    ACC = epi.tile([128, 1], F32, name="acc", tag="acc")
    nc.vector.reduce_sum(ACC[:], LC[:], axis=mybir.AxisListType.X)
    nc.sync.dma_start(out=out, in_=ACC[:])


def _trace_kernel(tc, nc, zjs, zis, out):
    # chunk-major views parameterized by span (blocks per load)
    def zjs_blk_view(blk0, nblks):
        v = zjs.rearrange(
            "(q g p two) f -> q p g (two f)", p=128, two=2, g=nblks * BLK_G
        )
        return v[blk0 // nblks]

    def zis_blk_view(blk0, nblks):
        v = zis.rearrange(
            "(q g p two) f -> q p g (two f)", p=128, two=2, g=nblks * BLK_G
        )
        return v[blk0 // nblks]

    GC = BLK_G * 512  # cols per block tile

    with (
        tc.tile_pool(name="consts", bufs=1) as consts,
        tc.tile_pool(name="loads", bufs=1) as loads,
        tc.tile_pool(name="tjp", bufs=2, space="PSUM") as tjp,
        tc.tile_pool(name="tipp", bufs=1, space="PSUM") as tipp,
        tc.tile_pool(name="tip", bufs=3) as tip,
        tc.tile_pool(name="prod", bufs=4) as prod,
        tc.tile_pool(name="sq", bufs=4) as sqp,
        tc.tile_pool(name="spa", bufs=1, space="PSUM") as spa,
        tc.tile_pool(name="epi", bufs=1) as epi,
    ):
        ident = consts.tile([128, 128], BF16, name="ident", tag="ident")
        masks.make_identity(nc, ident[:])
        ones = consts.tile([128, 1], BF16, name="ones", tag="ones")
        nc.gpsimd.memset(ones[:], 1.0)

        SP = spa.tile([128, GROUPS * N_VALS], F32, name="sp", tag="sp")
        S = epi.tile([128, GROUPS * N_VALS], F32, name="stats", tag="stats")

        # cast-loads span 2 blocks each, issued just-in-time inside the
        # transpose stage so the scheduler's in-flight DMA window follows
        # consumption order (up-front issue starves the XBARs)
        LDJS: dict = {}
        LDIS: dict = {}
        TIS: dict = {}

        def issue_pair_loads(pair):
            LDI = loads.tile([128, 2 * GC], BF16, name=f"ldi{pair}")
            nc.gpsimd.dma_start(
                out=LDI[:].rearrange("p (g f) -> p g f", g=2 * BLK_G),
                in_=zis_b2[pair],
            )
            LDJ = loads.tile([128, 2 * GC], BF16, name=f"ldj{pair}")
            nc.gpsimd.dma_start(
                out=LDJ[:].rearrange("p (g f) -> p g f", g=2 * BLK_G),
                in_=zjs_b2[pair],
            )
            for j in range(2):
                LDJS[2 * pair + j] = LDJ[:, j * GC : (j + 1) * GC]
                LDIS[2 * pair + j] = LDI[:, j * GC : (j + 1) * GC]

        def stage_transpose(blk):
            if blk % 2 == 0:
                issue_pair_loads(blk // 2)
            # zjs -> PSUM feature-major via PE: per group cols [a0 a1 b0 b1]
            TJ = tjp.tile([128, GC], BF16, name="tj")
            for g in range(BLK_G):
                for k in range(4):
                    sl = slice(512 * g + 128 * k, 512 * g + 128 * (k + 1))
                    nc.tensor.transpose(TJ[:, sl], LDJS[blk][:, sl], ident[:])
            # zis -> SBUF feature-major via DMA XBAR: out col-blocks [c0 c1 d0 d1]
            if blk % 2 == 0:
                TI2 = tip.tile([128, 2 * GC], BF16, name="ti")
                eng = nc.sync if blk % 4 == 0 else nc.scalar
                src = bass.AP(
                    LDIS[blk].tensor,
                    LDIS[blk].offset,
                    [tuple(LDIS[blk].ap[0]), (1, 2 * GC)],
                )
                eng.dma_start(
                    out=TI2[:].rearrange("p (k n) -> p k n", n=128),
                    in_=src,
                    transpose=True,
                )
                TIS[blk] = TI2[:, 0:GC]
                TIS[blk + 1] = TI2[:, GC : 2 * GC]
            return TJ, TIS[blk]

        def stage_consume(blk, TJ, TI):
            # half-views: [p, g, 2, 256] (vector-pair r x fused slice-cols)
            TJr = TJ[:].rearrange("p (g r w) -> p g r w", r=2, w=256)
            TIr = TI[:].rearrange("p (g r w) -> p g r w", r=2, w=256)
            TJg = TJ[:].rearrange("p (g w) -> p g w", w=512)
            TIg = TI[:].rearrange("p (g w) -> p g w", w=512)
            # TI with c/d swapped per group: [d0 d1 c0 c1] via negative stride
            ti_ap = TI[:]
            TIswap = bass.AP(
                ti_ap.tensor,
                ti_ap.offset + 256,
                [tuple(ti_ap.ap[0]), (512, BLK_G), (-256, 2), (1, 256)],
            )

            # products (bf16, DVE), one PSUM operand max:
            # P1 = TJ x TI        -> [ac0 ac1 bd0 bd1]
            P1 = prod.tile([128, GC], BF16, name="p1")
            nc.vector.tensor_mul(
                P1[:].rearrange("p (g w) -> p g w", w=512), TJg, TIg
            )
            # P2 = TJ x TI(swap)  -> [ad0 ad1 bc0 bc1]
            P2 = prod.tile([128, GC], BF16, name="p2")
            nc.vector.tensor_mul(
                P2[:].rearrange("p (g r w) -> p g r w", r=2, w=256),
                TJr,
                TIswap,
            )
            # a-slices to SBUF so ab has only one PSUM operand
            CPA = prod.tile([128, GC // 2], BF16, name="cpa")
            CPAv = CPA[:].rearrange("p (g w) -> p g w", w=256)
            nc.vector.tensor_copy(
                CPAv, TJr[:, :, 0:1, :].rearrange("p g o w -> p g (o w)")
            )
            # P3 = [ab0 ab1]
            P3 = prod.tile([128, GC // 2], BF16, name="p3")
            nc.vector.tensor_mul(
                P3[:].rearrange("p (g w) -> p g w", w=256),
                CPAv,
                TJr[:, :, 1:2, :].rearrange("p g o w -> p g (o w)"),
            )
            # P4 = [cd0 cd1]
            P4 = prod.tile([128, GC // 2], BF16, name="p4")
            nc.vector.tensor_mul(
                P4[:].rearrange("p (g w) -> p g w", w=256),
                TIr[:, :, 0:1, :].rearrange("p g o w -> p g (o w)"),
                TIr[:, :, 1:2, :].rearrange("p g o w -> p g (o w)"),
            )

            # squares (ACT): [aa0 aa1 bb0 bb1], [cc0 cc1 dd0 dd1]
            SQJ = sqp.tile([128, GC], BF16, name="sqj")
            nc.scalar.activation(SQJ[:], TJ[:], ACTF.Square)
            SQI = sqp.tile([128, GC], BF16, name="sqi")
            nc.scalar.activation(SQI[:], TI[:], ACTF.Square)

            # PE ones-matmul reductions into SP
            goff = blk * BLK_G
            # per stat t: (tile, base col-block index within group)
            stat_src = [
                (SQJ, 0, 512),  # na from [aa0 aa1]
                (SQJ, 2, 512),  # nb
                (SQI, 0, 512),  # nc
                (SQI, 2, 512),  # nd
                (P3, 0, 256),   # ab
                (P4, 0, 256),   # cd
                (P1, 0, 512),   # ac
                (P1, 2, 512),   # bd
                (P2, 0, 512),   # ad
                (P2, 2, 512),   # bc
            ]
            for g in range(BLK_G):
                for t, (src, kb, per_g) in enumerate(stat_src):
                    col = (goff + g) * N_VALS + t
                    base = per_g * g + 128 * kb
                    nc.tensor.matmul(
                        SP[:, col : col + 1],
                        src[:, base : base + 128],
                        ones[:, 0:1],
                        start=True,
                        stop=False,
                    )
                    nc.tensor.matmul(
                        SP[:, col : col + 1],
                        src[:, base + 128 : base + 256],
                        ones[:, 0:1],
                        start=False,
                        stop=True,
                    )

        # software pipeline with lookahead 2: transpose blocks k+1, k+2 are
        # issued before the reduce-matmuls of block k so the in-order PE
        # queue never stalls on DVE/ACT results
        LOOKAHEAD = 3
        half_cols = GROUPS // 2 * N_VALS
        dens = {}
        coss = {}
        tiles = {}
        for blk in range(min(LOOKAHEAD, N_BLKS)):
            tiles[blk] = stage_transpose(blk)
        for blk in range(N_BLKS):
            if blk + LOOKAHEAD < N_BLKS:
                tiles[blk + LOOKAHEAD] = stage_transpose(blk + LOOKAHEAD)
            stage_consume(blk, *tiles.pop(blk))

        nc.vector.tensor_copy(S[:], SP[:])
        den, cos = _epi_pre(tc, nc, epi, S[:], 0, full=True)
        _epi_post2(tc, nc, epi, den, cos, out)


_NC_CACHE = None


def _build_nc():
    global _NC_CACHE
    if _NC_CACHE is not None:
        return _NC_CACHE
    nc = bacc.Bacc(
        "TRN2",
        target_bir_lowering=False,
        debug=False,
        num_devices=N_CORES,
        dynamic_dma_scratch_size=49152,
        num_swdge_queues=4,
    )
    zjs = nc.dram_tensor("zjs", [ROWS, D], F32, kind="ExternalInput")
    zis = nc.dram_tensor("zis", [ROWS, D], F32, kind="ExternalInput")
    out = nc.dram_tensor("out", [128, 1], F32, kind="ExternalOutput")
    with tile.TileContext(nc) as tc:
        _trace_kernel(tc, nc, zjs.ap(), zis.ap(), out.ap())
    nc.compile()
    _NC_CACHE = nc
    return nc


def run_cores(zis, zjs, trace=False):
    nc = _build_nc()
    zis = np.ascontiguousarray(np.asarray(zis, dtype=np.float32))
    zjs = np.ascontiguousarray(np.asarray(zjs, dtype=np.float32))
    in_maps = []
    for i in range(N_CORES):
        sl = slice(i * ROWS, (i + 1) * ROWS)
        in_maps.append({"zis": zis[sl], "zjs": zjs[sl]})
    res = run_bass_kernel_spmd(nc, in_maps, list(range(N_CORES)), trace=trace)
    return [r["out"] for r in res.results], res


def kernel(zis, zjs):
    outs, _ = run_cores(zis, zjs, trace=False)
    total = np.sum([o.astype(np.float64).sum() for o in outs])
    return np.asarray(total / B_FULL, dtype=np.float32)


# revision 4
# speedup vs baseline: 1.0845x; 1.0420x over previous
"""Conditional NT-Xent loss kernel for Trainium2 (8 NeuronCores, SPMD data-parallel).

Per chunk t: a=zjs[2t], b=zjs[2t+1], c=zis[2t], d=zis[2t+1].
Needs 4 norms (na..nd) + 6 dots (ab cd ac bd ad bc), then per-row softmax math.

Pipeline per block of 4 groups (group = 128 chunks):
  - gpsimd cast-DMA f32->bf16 chunk-major loads (graded sizes, few DMA
    instructions - the tile scheduler serializes DMAs at ~2.3us each)
  - both tensors -> feature-major PSUM via PE transposes, software-pipelined
    ahead of the reduce-matmuls so the in-order PE queue never stalls
  - zis materialized PSUM->SBUF (alternating DVE/ACT) so every DVE product
    has at most one PSUM operand (hardware restriction) 
  - DVE bf16 products: TJxTI -> [ac|bd], TJxTI(neg-stride swap) -> [ad|bc],
    cd, and ab via an a-slice SBUF copy; ACT squares for the norms
  - PE ones-matmul column reductions (2 accumulating matmuls per stat) into
    one persistent PSUM stats bank
  - epilogue: DVE fast-inverse-sqrt (+1 Newton) for 1/sqrt(nx*ny) so ACT only
    needs Exp (table set 0, shared with Square) plus one final Ln (set 5)
Host sums the 8 cores' [128,1] partials / B.
"""

import numpy as np

import concourse.bass as bass
import concourse.tile as tile
from concourse import bacc, masks, mybir
from concourse.bass_utils import run_bass_kernel_spmd

N_CORES = 8
B_FULL = 65536
ROWS = B_FULL // N_CORES  # 8192
D = 256
GROUPS = ROWS // 256      # 32 groups of 128 chunks
BLK_G = 4                 # groups per pipeline block
N_BLKS = GROUPS // BLK_G
HALF_BLKS = N_BLKS // 2
F32 = mybir.dt.float32
BF16 = mybir.dt.bfloat16
FP8 = mybir.dt.float8e4
ALU = mybir.AluOpType
ACTF = mybir.ActivationFunctionType

N_VALS = 10  # stat col order per group: na nb nc nd ab cd ac bd ad bc


MAGIC_F = float(np.frombuffer(np.uint32(0x5F3759DF).tobytes(), dtype=np.float32)[0])
I32 = mybir.dt.int32


def _epi_pre(tc, nc, epi, S, half, full=False):
    """Epilogue up to the denominators: DVE math + one ACT Exp
    (table set 0, same as Square - safe to interleave with the main loop)."""
    G = GROUPS if full else GROUPS // 2
    Sv = S.rearrange("p (g t) -> p g t", t=N_VALS)
    svals = Sv[:, :, 4:10]

    # RP_xy = nx * ny (pair order ab cd ac bd ad bc)
    RP = epi.tile([128, G * 6], F32, name=f"rp{half}", tag=f"rp{half}")
    RPv = RP[:].rearrange("p (g t) -> p g t", t=6)
    pair_norm_idx = [(0, 1), (2, 3), (0, 2), (1, 3), (0, 3), (1, 2)]
    for t, (x, y) in enumerate(pair_norm_idx):
        nc.vector.tensor_mul(
            RPv[:, :, t : t + 1], Sv[:, :, x : x + 1], Sv[:, :, y : y + 1]
        )

    # rr = rsqrt(RP) via fast-inverse-sqrt + 1 Newton iteration (DVE only)
    MAG = epi.tile([128, G * 6], F32, name=f"mag{half}", tag=f"mag{half}")
    nc.gpsimd.memset(MAG[:], MAGIC_F)
    C15 = epi.tile([128, G * 6], F32, name=f"c15{half}", tag=f"c15{half}")
    nc.gpsimd.memset(C15[:], 1.5)
    Y = epi.tile([128, G * 6], F32, name=f"y{half}", tag=f"y{half}")
    SH = epi.tile([128, G * 6], F32, name=f"sh{half}", tag=f"sh{half}")
    nc.vector.tensor_scalar(
        out=SH[:].bitcast(I32),
        in0=RP[:].bitcast(I32),
        scalar1=1,
        scalar2=None,
        op0=ALU.logical_shift_right,
    )
    nc.vector.tensor_tensor(
        out=Y[:].bitcast(I32),
        in0=MAG[:].bitcast(I32),
        in1=SH[:].bitcast(I32),
        op=ALU.subtract,
    )
    T2 = epi.tile([128, G * 6], F32, name=f"t2{half}", tag=f"t2{half}")
    T3 = epi.tile([128, G * 6], F32, name=f"t3{half}", tag=f"t3{half}")
    nc.vector.tensor_mul(T2[:], Y[:], Y[:])
    nc.vector.tensor_mul(T2[:], T2[:], RP[:])
    nc.vector.scalar_tensor_tensor(
        out=T3[:], in0=T2[:], scalar=-0.5, in1=C15[:], op0=ALU.mult, op1=ALU.add
    )
    nc.vector.tensor_mul(Y[:], Y[:], T3[:])

    # cos_xy = s_xy * rr_xy
    C = epi.tile([128, G * 6], F32, name=f"cos{half}", tag=f"cos{half}")
    Cv = C[:].rearrange("p (g t) -> p g t", t=6)
    nc.vector.tensor_mul(Cv, svals, Y[:].rearrange("p (g t) -> p g t", t=6))

    # E_xy = exp(2 cos)  (ACT set 0, same as Square)
    E = epi.tile([128, G * 6], F32, name=f"e{half}", tag=f"e{half}")
    Ev = E[:].rearrange("p (g t) -> p g t", t=6)
    nc.scalar.activation(Ev, Cv, ACTF.Exp, scale=2.0)

    def ecol(t):
        return Ev[:, :, t : t + 1]

    # softmax denominators for the 4 rows of each chunk
    DEN = epi.tile([128, G * 4], F32, name=f"den{half}", tag=f"den{half}")
    DENv = DEN[:].rearrange("p (g t) -> p g t", t=4)
    TMP = epi.tile([128, G * 4], F32, name=f"tmp{half}", tag=f"tmp{half}")
    TMPv = TMP[:].rearrange("p (g t) -> p g t", t=4)
    den_terms = [
        (0, 4, 2),  # D0 = (ab + ad) + ac
        (0, 5, 3),  # D1 = (ab + bc) + bd
        (5, 1, 2),  # D2 = (bc + cd) + ac
        (4, 1, 3),  # D3 = (ad + cd) + bd
    ]
    for r, (u, v, w) in enumerate(den_terms):
        nc.vector.tensor_add(TMPv[:, :, r : r + 1], ecol(u), ecol(v))
        nc.vector.tensor_add(DENv[:, :, r : r + 1], TMPv[:, :, r : r + 1], ecol(w))
    return DEN, C


def _epi_post2(tc, nc, epi, den, cos, out):
    """Final Ln (single ACT set switch) + loss accumulation."""
    LD = epi.tile([128, GROUPS * 4], F32, name="ld", tag="ld")
    nc.scalar.activation(LD[:], den[:], ACTF.Ln)

    LG = epi.tile([128, GROUPS], F32, name="lg", tag="lg")
    nc.vector.reduce_sum(
        LG[:].rearrange("p (g o) -> p g o", o=1),
        LD[:].rearrange("p (g t) -> p g t", t=4),
        axis=mybir.AxisListType.X,
    )

    T1 = epi.tile([128, GROUPS], F32, name="t1", tag="t1")
    T1v = T1[:].rearrange("p (g o) -> p g o", o=1)
    Cv = cos[:].rearrange("p (g t) -> p g t", t=6)
    nc.vector.tensor_add(T1v[:], Cv[:, :, 2:3], Cv[:, :, 3:4])

    # loss per chunk-col = LG - 4*T1
    LC = epi.tile([128, GROUPS], F32, name="lc", tag="lc")
    nc.vector.scalar_tensor_tensor(
        out=LC[:], in0=T1[:], scalar=-4.0, in1=LG[:], op0=ALU.mult, op1=ALU.add
    )

    ACC = epi.tile([128, 1], F32, name="acc", tag="acc")
    nc.vector.reduce_sum(ACC[:], LC[:], axis=mybir.AxisListType.X)
    nc.sync.dma_start(out=out, in_=ACC[:])


def _trace_kernel(tc, nc, zjs, zis, out):
    # chunk-major views parameterized by span (blocks per load)
    def zjs_blk_view(blk0, nblks):
        v = zjs.rearrange(
            "(q g p two) f -> q p g (two f)", p=128, two=2, g=nblks * BLK_G
        )
        return v[blk0 // nblks]

    def zis_blk_view(blk0, nblks):
        v = zis.rearrange(
            "(q g p two) f -> q p g (two f)", p=128, two=2, g=nblks * BLK_G
        )
        return v[blk0 // nblks]

    GC = BLK_G * 512  # cols per block tile

    with (
        tc.tile_pool(name="consts", bufs=1) as consts,
        tc.tile_pool(name="loads", bufs=1) as loads,
        tc.tile_pool(name="tjp", bufs=2, space="PSUM") as tjp,
        tc.tile_pool(name="tipp", bufs=1, space="PSUM") as tipp,
        tc.tile_pool(name="tip", bufs=3) as tip,
        tc.tile_pool(name="prod", bufs=4) as prod,
        tc.tile_pool(name="sq", bufs=4) as sqp,
        tc.tile_pool(name="spa", bufs=1, space="PSUM") as spa,
        tc.tile_pool(name="epi", bufs=1) as epi,
    ):
        ident = consts.tile([128, 128], BF16, name="ident", tag="ident")
        masks.make_identity(nc, ident[:])
        ones = consts.tile([128, 1], BF16, name="ones", tag="ones")
        nc.gpsimd.memset(ones[:], 1.0)

        SP = spa.tile([128, GROUPS * N_VALS], F32, name="sp", tag="sp")
        S = epi.tile([128, GROUPS * N_VALS], F32, name="stats", tag="stats")

        # cast-loads span 2 blocks each, issued just-in-time inside the
        # transpose stage so the scheduler's in-flight DMA window follows
        # consumption order (up-front issue starves the XBARs)
        LDJS: dict = {}
        LDIS: dict = {}
        TIS: dict = {}

        def issue_pair_loads(pair):
            LDI = loads.tile([128, 2 * GC], BF16, name=f"ldi{pair}")
            nc.gpsimd.dma_start(
                out=LDI[:].rearrange("p (g f) -> p g f", g=2 * BLK_G),
                in_=zis_b2[pair],
            )
            LDJ = loads.tile([128, 2 * GC], BF16, name=f"ldj{pair}")
            nc.gpsimd.dma_start(
                out=LDJ[:].rearrange("p (g f) -> p g f", g=2 * BLK_G),
                in_=zjs_b2[pair],
            )
            for j in range(2):
                LDJS[2 * pair + j] = LDJ[:, j * GC : (j + 1) * GC]
                LDIS[2 * pair + j] = LDI[:, j * GC : (j + 1) * GC]

        def stage_transpose(blk):
            if blk % 2 == 0:
                issue_pair_loads(blk // 2)
            # zjs -> PSUM feature-major via PE: per group cols [a0 a1 b0 b1]
            TJ = tjp.tile([128, GC], BF16, name="tj")
            for g in range(BLK_G):
                for k in range(4):
                    sl = slice(512 * g + 128 * k, 512 * g + 128 * (k + 1))
                    nc.tensor.transpose(TJ[:, sl], LDJS[blk][:, sl], ident[:])
            # zis -> SBUF feature-major via DMA XBAR: out col-blocks [c0 c1 d0 d1]
            if blk % 2 == 0:
                TI2 = tip.tile([128, 2 * GC], BF16, name="ti")
                eng = nc.sync if blk % 4 == 0 else nc.scalar
                src = bass.AP(
                    LDIS[blk].tensor,
                    LDIS[blk].offset,
                    [tuple(LDIS[blk].ap[0]), (1, 2 * GC)],
                )
                eng.dma_start(
                    out=TI2[:].rearrange("p (k n) -> p k n", n=128),
                    in_=src,
                    transpose=True,
                )
                TIS[blk] = TI2[:, 0:GC]
                TIS[blk + 1] = TI2[:, GC : 2 * GC]
            return TJ, TIS[blk]

        def stage_consume(blk, TJ, TI):
            # half-views: [p, g, 2, 256] (vector-pair r x fused slice-cols)
            TJr = TJ[:].rearrange("p (g r w) -> p g r w", r=2, w=256)
            TIr = TI[:].rearrange("p (g r w) -> p g r w", r=2, w=256)
            TJg = TJ[:].rearrange("p (g w) -> p g w", w=512)
            TIg = TI[:].rearrange("p (g w) -> p g w", w=512)
            # TI with c/d swapped per group: [d0 d1 c0 c1] via negative stride
            ti_ap = TI[:]
            TIswap = bass.AP(
                ti_ap.tensor,
                ti_ap.offset + 256,
                [tuple(ti_ap.ap[0]), (512, BLK_G), (-256, 2), (1, 256)],
            )

            # products (bf16, DVE), one PSUM operand max:
            # P1 = TJ x TI        -> [ac0 ac1 bd0 bd1]
            P1 = prod.tile([128, GC], BF16, name="p1")
            nc.vector.tensor_mul(
                P1[:].rearrange("p (g w) -> p g w", w=512), TJg, TIg
            )
            # P2 = TJ x TI(swap)  -> [ad0 ad1 bc0 bc1]
            P2 = prod.tile([128, GC], BF16, name="p2")
            nc.vector.tensor_mul(
                P2[:].rearrange("p (g r w) -> p g r w", r=2, w=256),
                TJr,
                TIswap,
            )
            # a-slices to SBUF so ab has only one PSUM operand
            CPA = prod.tile([128, GC // 2], BF16, name="cpa")
            CPAv = CPA[:].rearrange("p (g w) -> p g w", w=256)
            nc.vector.tensor_copy(
                CPAv, TJr[:, :, 0:1, :].rearrange("p g o w -> p g (o w)")
            )
            # P3 = [ab0 ab1]
            P3 = prod.tile([128, GC // 2], BF16, name="p3")
            nc.vector.tensor_mul(
                P3[:].rearrange("p (g w) -> p g w", w=256),
                CPAv,
                TJr[:, :, 1:2, :].rearrange("p g o w -> p g (o w)"),
            )
            # P4 = [cd0 cd1]
            P4 = prod.tile([128, GC // 2], BF16, name="p4")
            nc.vector.tensor_mul(
                P4[:].rearrange("p (g w) -> p g w", w=256),
                TIr[:, :, 0:1, :].rearrange("p g o w -> p g (o w)"),
                TIr[:, :, 1:2, :].rearrange("p g o w -> p g (o w)"),
            )

            # squares (ACT): [aa0 aa1 bb0 bb1], [cc0 cc1 dd0 dd1]
            SQJ = sqp.tile([128, GC], BF16, name="sqj")
            nc.scalar.activation(SQJ[:], TJ[:], ACTF.Square)
            SQI = sqp.tile([128, GC], BF16, name="sqi")
            nc.scalar.activation(SQI[:], TI[:], ACTF.Square)

            # PE ones-matmul reductions into SP
            goff = blk * BLK_G
            # per stat t: (tile, base col-block index within group)
            stat_src = [
                (SQJ, 0, 512),  # na from [aa0 aa1]
                (SQJ, 2, 512),  # nb
                (SQI, 0, 512),  # nc
                (SQI, 2, 512),  # nd
                (P3, 0, 256),   # ab
                (P4, 0, 256),   # cd
                (P1, 0, 512),   # ac
                (P1, 2, 512),   # bd
                (P2, 0, 512),   # ad
                (P2, 2, 512),   # bc
            ]
            for g in range(BLK_G):
                for t, (src, kb, per_g) in enumerate(stat_src):
                    col = (goff + g) * N_VALS + t
                    base = per_g * g + 128 * kb
                    nc.tensor.matmul(
                        SP[:, col : col + 1],
                        src[:, base : base + 128],
                        ones[:, 0:1],
                        start=True,
                        stop=False,
                    )
                    nc.tensor.matmul(
                        SP[:, col : col + 1],
                        src[:, base + 128 : base + 256],
                        ones[:, 0:1],
                        start=False,
                        stop=True,
                    )

        # software pipeline with lookahead 2: transpose blocks k+1, k+2 are
        # issued before the reduce-matmuls of block k so the in-order PE
        # queue never stalls on DVE/ACT results
        LOOKAHEAD = 3
        half_cols = GROUPS // 2 * N_VALS
        dens = {}
        coss = {}
        tiles = {}
        for blk in range(min(LOOKAHEAD, N_BLKS)):
            tiles[blk] = stage_transpose(blk)
        for blk in range(N_BLKS):
            if blk + LOOKAHEAD < N_BLKS:
                tiles[blk + LOOKAHEAD] = stage_transpose(blk + LOOKAHEAD)
            stage_consume(blk, *tiles.pop(blk))

        nc.vector.tensor_copy(S[:], SP[:])
        den, cos = _epi_pre(tc, nc, epi, S[:], 0, full=True)
        _epi_post2(tc, nc, epi, den, cos, out)


_NC_CACHE = None


def _build_nc():
    global _NC_CACHE
    if _NC_CACHE is not None:
        return _NC_CACHE
    nc = bacc.Bacc(
        "TRN2",
        target_bir_lowering=False,
        debug=False,
        num_devices=N_CORES,
        dynamic_dma_scratch_size=49152,
        num_swdge_queues=4,
    )
    zjs = nc.dram_tensor("zjs", [ROWS, D], F32, kind="ExternalInput")
    zis = nc.dram_tensor("zis", [ROWS, D], F32, kind="ExternalInput")
    out = nc.dram_tensor("out", [128, 1], F32, kind="ExternalOutput")
    with tile.TileContext(nc) as tc:
        _trace_kernel(tc, nc, zjs.ap(), zis.ap(), out.ap())
    nc.compile()
    _NC_CACHE = nc
    return nc


def run_cores(zis, zjs, trace=False):
    nc = _build_nc()
    zis = np.ascontiguousarray(np.asarray(zis, dtype=np.float32))
    zjs = np.ascontiguousarray(np.asarray(zjs, dtype=np.float32))
    in_maps = []
    for i in range(N_CORES):
        sl = slice(i * ROWS, (i + 1) * ROWS)
        in_maps.append({"zis": zis[sl], "zjs": zjs[sl]})
    res = run_bass_kernel_spmd(nc, in_maps, list(range(N_CORES)), trace=trace)
    return [r["out"] for r in res.results], res


def kernel(zis, zjs):
    outs, _ = run_cores(zis, zjs, trace=False)
    total = np.sum([o.astype(np.float64).sum() for o in outs])
    return np.asarray(total / B_FULL, dtype=np.float32)
